# revision 1
# baseline (speedup 1.0000x reference)
"""Depthwise 9x9 same-padding conv (single shared kernel) on Trainium2.

Strategy (per NeuronCore, pure data-parallel over batch across 8 cores):
  - Treat each (b, c) image [256, 256] independently; 256 images per core.
  - Contract over image ROWS on the PE: for each horizontal tap v (9), one
    matmul with a banded Toeplitz weight matrix (built host-side from K)
    accumulating into PSUM:  out[i, j] += sum_u K[u, v] * X[i+u-4, j+v-4].
    The horizontal shift j+v-4 is an AP offset into a width-padded SBUF
    tile; the vertical band lives in the stationary lhsT.
  - A 256-row image splits into two 128-row SBUF tiles. Rows 0..123 come
    entirely from tile0 (top-clipped band), rows 132..255 from tile1
    (bottom-clipped band). The 8 cross-tile rows 124..131 of 8 images are
    batched into one [128, W] strip tile with block-diagonal band weights.
  - J images are packed per DMA/SBUF tile (host pre-transposed layout) so
    every main DMA is one large contiguous 2D transfer. Input DMAs ride the
    SP HWDGE ring, output DMAs the ACT HWDGE ring, edge DMAs SWDGE.
"""

import numpy as np
import ml_dtypes

import concourse.bass as bass
from concourse import bacc
import concourse.mybir as mybir
import concourse.tile as tile
from concourse.bass_utils import run_bass_kernel_spmd

N_CORES = 8
B, C, H, W = 32, 64, 256, 256
KS, PAD = 9, 4
BC = B // N_CORES          # batches per core
NIMG = BC * C              # images per core
WP = W + 2 * PAD           # padded width 264
MT = 124                   # main out-rows per half-tile (0..123 / 132..255)
EG = 8                     # images per edge-strip group
NGRP = NIMG // EG
J = 4                      # images packed per main DMA / SBUF tile
NBLK = NIMG // J

# float32r inputs/weights: full fp32 storage, PE streams it at bf16 rate for
# N>=256 (relaxed-precision matmul), fp32 PSUM accumulation + fp32 output.
IN_DT = mybir.dt.float32r
IN_NP = np.float32

LAST_RESULT = None         # test.py inspects this


def _build_weights(Kf):
    """Banded Toeplitz lhsT matrices from the 9x9 kernel Kf (float32).

    Wtop[v, i', i] = Kf[i'-i+4, v]   out rows 0..123   from X rows 0..127
    Wbot[v, i', m] = Kf[i'-m,   v]   out rows 132..255 from X rows 128..255
    Wedge[v, 16g+m+u, 8g+m] = Kf[u, v]  out rows 124..131 from X rows 120..135,
                                         8 images block-diagonal
    """
    Wtop = np.zeros((KS, 128, MT), np.float32)
    Wbot = np.zeros((KS, 128, MT), np.float32)
    Wedge = np.zeros((KS, 128, 8 * EG), np.float32)
    for v in range(KS):
        for i in range(MT):
            for u in range(KS):
                ip = i + u - PAD
                if 0 <= ip < 128:
                    Wtop[v, ip, i] = Kf[u, v]
                ipb = i + u
                if 0 <= ipb < 128:
                    Wbot[v, ipb, i] = Kf[u, v]
        for g in range(EG):
            for m in range(8):
                for u in range(KS):
                    Wedge[v, 16 * g + m + u, 8 * g + m] = Kf[u, v]
    return Wtop, Wbot, Wedge


def _build_nc(n_img=NIMG, xbufs=4, obufs=4, psbufs=5):
    n_blk = n_img // J
    n_grp = n_img // EG
    nc = bacc.Bacc("TRN2", target_bir_lowering=False)
    Xm = nc.dram_tensor("Xm", [n_blk, 2, 128, J * WP], IN_DT, kind="ExternalInput")
    Xe = nc.dram_tensor("Xe", [n_grp, 128, WP], IN_DT, kind="ExternalInput")
    Wt = nc.dram_tensor("Wt", [KS, 128, MT], IN_DT, kind="ExternalInput")
    Wb = nc.dram_tensor("Wb", [KS, 128, MT], IN_DT, kind="ExternalInput")
    We = nc.dram_tensor("We", [KS, 128, 8 * EG], IN_DT, kind="ExternalInput")
    Om = nc.dram_tensor(
        "Om", [n_blk, 2, MT, J * W], mybir.dt.float32, kind="ExternalOutput"
    )
    Oe = nc.dram_tensor(
        "Oe", [n_grp, 8 * EG, W], mybir.dt.float32, kind="ExternalOutput"
    )

    with tile.TileContext(nc) as tc:
        with (
            tc.tile_pool(name="wpool", bufs=1) as wpool,
            tc.tile_pool(name="xpool", bufs=xbufs) as xpool,
            tc.tile_pool(name="epool", bufs=2) as epool,
            tc.tile_pool(name="opool", bufs=obufs) as opool,
            tc.tile_pool(name="oepool", bufs=2) as oepool,
            tc.tile_pool(name="psum", bufs=psbufs, space="PSUM") as pspool,
            tc.tile_pool(name="psum_e", bufs=2, space="PSUM") as pepool,
        ):
            wt = wpool.tile([128, KS, MT], IN_DT)
            wb = wpool.tile([128, KS, MT], IN_DT)
            we = wpool.tile([128, KS, 8 * EG], IN_DT)
            nc.gpsimd.dma_start(out=wt[:], in_=Wt[:].rearrange("v p m -> p v m"))
            nc.gpsimd.dma_start(out=wb[:], in_=Wb[:].rearrange("v p m -> p v m"))
            nc.gpsimd.dma_start(out=we[:], in_=We[:].rearrange("v p m -> p v m"))

            for blk in range(n_blk):
                for half in range(2):
                    xt = xpool.tile([128, J * WP], IN_DT)
                    nc.sync.dma_start(out=xt[:], in_=Xm[blk, half])
                    ot = opool.tile([MT, J * W], mybir.dt.float32)
                    wsel = wt if half == 0 else wb
                    for j in range(J):
                        ps = pspool.tile([MT, W], mybir.dt.float32)
                        for v in range(KS):
                            nc.tensor.matmul(
                                ps[:],
                                wsel[:, v, :],
                                xt[:, j * WP + v : j * WP + v + W],
                                start=(v == 0),
                                stop=(v == KS - 1),
                            )
                        nc.vector.tensor_copy(ot[:, j * W : (j + 1) * W], ps[:])
                    nc.scalar.dma_start(out=Om[blk, half], in_=ot[:])

                if blk % (EG // J) == 0:
                    g = blk // (EG // J)
                    et = epool.tile([128, WP], IN_DT)
                    nc.gpsimd.dma_start(out=et[:], in_=Xe[g])
                    pse = pepool.tile([8 * EG, W], mybir.dt.float32)
                    for v in range(KS):
                        nc.tensor.matmul(
                            pse[:],
                            we[:, v, :],
                            et[:, v : v + W],
                            start=(v == 0),
                            stop=(v == KS - 1),
                        )
                    oe = oepool.tile([8 * EG, W], mybir.dt.float32)
                    nc.vector.tensor_copy(oe[:], pse[:])
                    nc.gpsimd.dma_start(out=Oe[g], in_=oe[:])
    nc.compile()
    return nc


def _prep_inputs(X):
    """Host prep: pad width, cast bf16, pack J images per tile row-block."""
    Xp = np.zeros((B * C, H, WP), IN_NP)
    Xp[:, :, PAD : PAD + W] = X.reshape(B * C, H, W)
    # main: [cores, blk, J, 2half, 128, WP] -> [cores, blk, 2, 128, J, WP]
    Xm = (
        Xp.reshape(N_CORES, NBLK, J, 2, 128, WP)
        .transpose(0, 1, 3, 4, 2, 5)
        .reshape(N_CORES, NBLK, 2, 128, J * WP)
    )
    Xm = np.ascontiguousarray(Xm)
    # edge strips: rows 120..135 of each image, 8 images stacked per group
    Xe = np.ascontiguousarray(
        Xp[:, 120:136, :].reshape(N_CORES, NGRP, 128, WP)
    )
    return Xm, Xe


def _assemble_output(res):
    """Reassemble [B, C, H, W] fp32 from per-core Om/Oe."""
    out = np.empty((N_CORES, NIMG, H, W), np.float32)
    for k in range(N_CORES):
        om = res.results[k]["Om"].reshape(NBLK, 2, MT, J, W)
        oe = res.results[k]["Oe"].reshape(NGRP * EG, 8, W)
        o = out[k].reshape(NBLK, J, H, W)
        o[:, :, 0:MT, :] = om[:, 0].transpose(0, 2, 1, 3)
        o[:, :, 132 : 132 + MT, :] = om[:, 1].transpose(0, 2, 1, 3)
        out[k][:, 124:132, :] = oe
    return out.reshape(B, C, H, W)


def kernel(X, K):
    global LAST_RESULT
    X = np.asarray(X)
    K = np.asarray(K)
    assert X.shape == (B, C, H, W) and K.shape == (1, 1, KS, KS)

    Xm, Xe = _prep_inputs(X)
    Wtop, Wbot, Wedge = _build_weights(K[0, 0].astype(np.float32))
    Wtop = Wtop.astype(IN_NP)
    Wbot = Wbot.astype(IN_NP)
    Wedge = Wedge.astype(IN_NP)

    nc = _build_nc()
    in_maps = [
        {"Xm": Xm[k], "Xe": Xe[k], "Wt": Wtop, "Wb": Wbot, "We": Wedge}
        for k in range(N_CORES)
    ]
    res = run_bass_kernel_spmd(nc, in_maps, core_ids=list(range(N_CORES)))
    LAST_RESULT = res
    return _assemble_output(res)

